# revision 1
# baseline (speedup 1.0000x reference)
"""Trainium2 Bass kernel for nn_LocalAggregator (GNN message passing).

Computation (reference semantics):
    te    = p0*exp(-t) + p1
    h     = [hidden[..., :127] | te]
    e_k   = leaky_relu((h*a_k) @ h^T, 0.2)          k = 0..3
    alpha = softmax(select_by_adj(e_k, adj, -inf))   over last axis
    out   = alpha @ h

Device strategy (pure data-parallel over batch, 8 cores x 8 batches):
  - Score planes e_k are symmetric bilinear forms, so we compute them in
    [j, i] layout (neighbor j on partitions).  The softmax denominator
    (sum over j) then falls out of the aggregation matmul as an extra
    ones-column of h -- no transposes, no reductions.
  - The 4-way adjacency select runs as a copy_predicated ladder over the
    PSUM score planes using host-shipped uint8 bit-plane masks
    (b0 = lsb(adj-1), b1 = adj>=3); the adj==0 kill is a 2x-rate
    tensor_tensor add of a host-shipped -1024 plane before exp.
  - Prelu (leaky relu, alpha honored) and Exp run on the scalar engine;
    per-row 1/Z normalization rides the output copy as an ACT scale.
  - All per-batch inputs arrive as two packed DMAs (bf16 blob + u8 blob)
    and leave as one DMA, keeping the SP sequencer off the critical path.
"""

import os
import sys

import numpy as np

for _p in ("/opt/trn_rl_repo", "/root/.axon_site/_ro/trn_rl_repo"):
    if os.path.isdir(_p) and _p not in sys.path:
        sys.path.insert(0, _p)

B, N, DIM = 64, 512, 128
NCORES = 8
BPC = B // NCORES          # batches per core
JC = N // 128              # j-chunks per batch
IC = N // 128              # i-chunks per batch
HAUG = 132                 # 128 dims + ones col + pad
LEAKY_ALPHA = 0.2
ZKILL = -1024.0

# packed bf16 blob offsets (per-partition free-dim layout)
OFF_HT = 0                     # hT            [128, N]      (blob A)
OFF_HTK = N                    # hTk (4x)      [128, 4, N]   (blob A)
BINAW = 5 * N
OFF_HAUG = 0                   # haug          [128, JC, HAUG]  (blob B)
OFF_ZNEG = JC * HAUG           # zneg          [128, JC, N]     (blob B)
BINBW = JC * HAUG + JC * N
MMW = 2 * JC * N               # u8 blob: b0m then b1m

_CACHE = {}


def _build_nc(repeat=1):
    import concourse.bass as bass
    from concourse import bacc, mybir
    from concourse.tile import TileContext

    bf16 = mybir.dt.bfloat16
    f32 = mybir.dt.float32
    u8 = mybir.dt.uint8
    act = mybir.ActivationFunctionType

    nc = bacc.Bacc(None, target_bir_lowering=False)

    bina_d = nc.declare_dram_parameter("bina", [BPC, 128, BINAW], bf16, isOutput=False)
    mm_d = nc.declare_dram_parameter("mm", [BPC, 128, MMW], u8, isOutput=False)
    binb_d = nc.declare_dram_parameter("binb", [BPC, 128, BINBW], bf16, isOutput=False)
    out_d = nc.declare_dram_parameter("out", [BPC, 128, IC, DIM], f32, isOutput=True)

    with TileContext(nc) as tc:
        with (
            tc.tile_pool(name="inp", bufs=3) as inp,
            tc.tile_pool(name="work", bufs=6) as work,
            tc.tile_pool(name="outp", bufs=3) as outp,
            tc.tile_pool(name="npool", bufs=8) as npool,
            tc.tile_pool(name="pse", bufs=6, space=bass.MemorySpace.PSUM) as pse,
            tc.tile_pool(name="psa", bufs=2, space=bass.MemorySpace.PSUM) as psa,
        ):
            for rep, b in [(r, bb) for r in range(repeat) for bb in range(BPC)]:
                bina1_t = inp.tile([128, 3 * N], bf16)
                mm_t = inp.tile([128, MMW], u8)
                bina2_t = inp.tile([128, 2 * N], bf16)
                binb_t = inp.tile([128, BINBW], bf16)
                nc.sync.dma_start(out=bina1_t[:], in_=bina_d[b, :, 0:3 * N])
                nc.sync.dma_start(out=mm_t[:], in_=mm_d[b])
                nc.sync.dma_start(out=bina2_t[:], in_=bina_d[b, :, 3 * N:])
                nc.sync.dma_start(out=binb_t[:], in_=binb_d[b])

                hT = bina1_t[:, OFF_HT:OFF_HT + N]

                ns = []
                for jc in range(JC):
                    e = [pse.tile([128, N], f32, tag="e", name=f"e{rep}_{b}_{jc}_{k}")
                         for k in range(4)]
                    for k in range(4):
                        # e_k[j, i] = sum_d hT[d, j-chunk] * (a_k . h)^T[d, i]
                        nc.tensor.matmul(
                            e[k][:],
                            hT[:, jc * 128:(jc + 1) * 128],
                            (bina1_t[:, (1 + k) * N:(2 + k) * N] if k < 2
                             else bina2_t[:, (k - 2) * N:(k - 1) * N]),
                            start=True,
                            stop=True,
                        )

                    b0m = mm_t[:, jc * N:(jc + 1) * N]
                    b1m = mm_t[:, (JC + jc) * N:(JC + jc + 1) * N]

                    # 4-way select ladder -> e[0] holds e_{adj-1}
                    nc.vector.copy_predicated(e[0][:], b0m, e[1][:])
                    nc.vector.copy_predicated(e[2][:], b0m, e[3][:])
                    nc.vector.copy_predicated(e[0][:], b1m, e[2][:])

                    npre = work.tile([128, N], bf16)
                    nc.scalar.activation(
                        npre[:], e[0][:], act.Prelu, alpha=LEAKY_ALPHA
                    )
                    # nmask = npre + zneg   (adj==0 -> exp == 0)
                    nmask = work.tile([128, N], bf16)
                    nc.vector.tensor_add(
                        nmask[:],
                        binb_t[:, OFF_ZNEG + jc * N:OFF_ZNEG + (jc + 1) * N],
                        npre[:],
                    )
                    n = npool.tile([128, N], bf16, tag="n", name=f"n{rep}_{b}_{jc}")
                    nc.scalar.activation(n[:], nmask[:], act.Exp)
                    ns.append(n)

                outt = outp.tile([128, IC, DIM], f32, tag="outt",
                                 name=f"outt{rep}_{b}")
                for ic in range(IC):
                    agg = psa.tile([128, HAUG], f32, tag="agg", name=f"agg{rep}_{b}_{ic}")
                    for jc in range(JC):
                        # out_un[i-chunk, 0:129] += n[:, i-chunk].T @ [h | 1]
                        nc.tensor.matmul(
                            agg[:, 0:DIM + 1],
                            ns[jc][:, ic * 128:(ic + 1) * 128],
                            binb_t[:, OFF_HAUG + jc * HAUG:OFF_HAUG + jc * HAUG + DIM + 1],
                            start=(jc == 0),
                            stop=(jc == JC - 1),
                        )
                    rz = outp.tile([128, 1], f32, tag="rz", name=f"rz{rep}_{b}_{ic}")
                    nc.vector.reciprocal(rz[:], agg[:, DIM:DIM + 1])
                    nc.scalar.activation(
                        outt[:, ic, :], agg[:, 0:DIM], act.Copy, scale=rz[:]
                    )
                nc.sync.dma_start(out=out_d[b], in_=outt[:])

    nc.compile()
    return nc


def _get_nc():
    if "nc" not in _CACHE:
        _CACHE["nc"] = _build_nc()
    return _CACHE["nc"]


def _host_prep(hidden, adj, input_times, a0, a1, a2, a3, p0, p1):
    import ml_dtypes

    bf16 = ml_dtypes.bfloat16

    hidden = np.asarray(hidden, dtype=np.float32)
    adj = np.asarray(adj)
    input_times = np.asarray(input_times, dtype=np.float32)

    te = np.asarray(p0, np.float32) * np.exp(-input_times) + np.asarray(p1, np.float32)
    h = np.concatenate([hidden[:, :, :-1], te[:, :, None]], axis=2)      # [B,N,128] f32

    hT = np.swapaxes(h, 1, 2)                                            # [B,128,N]
    A = np.stack([a0, a1, a2, a3], 0).astype(np.float32)                 # [4,128]

    bina = np.zeros((B, 128, BINAW), bf16)
    bina[:, :, OFF_HT:OFF_HT + N] = hT.astype(bf16)
    for k in range(4):
        bina[:, :, OFF_HTK + k * N:OFF_HTK + (k + 1) * N] = \
            (A[k][None, :, None] * hT).astype(bf16)
    binb = np.zeros((B, 128, BINBW), bf16)

    # haug[b, jp, jc, c] = h[b, jc*128+jp, c] (+ ones col)
    haug = np.zeros((B, N, HAUG), np.float32)
    haug[:, :, :DIM] = h
    haug[:, :, DIM] = 1.0
    haug = haug.reshape(B, JC, 128, HAUG).transpose(0, 2, 1, 3)
    binb[:, :, OFF_HAUG:OFF_HAUG + JC * HAUG] = \
        haug.reshape(B, 128, JC * HAUG).astype(bf16)

    def chunkT(m):
        # mask[b, i, j] -> transposed + chunked [b, jp, jc*N + i]
        mT = np.swapaxes(m, 1, 2)
        return mT.reshape(B, JC, 128, N).transpose(0, 2, 1, 3).reshape(B, 128, JC * N)

    zneg = np.where(adj == 0, np.float32(ZKILL), np.float32(0.0))
    binb[:, :, OFF_ZNEG:] = chunkT(zneg).astype(bf16)

    mmb = np.zeros((B, 128, MMW), np.uint8)
    b0 = (((adj - 1) & 1) * (adj > 0)).astype(np.uint8)
    b1 = (adj >= 3).astype(np.uint8)
    mmb[:, :, :JC * N] = chunkT(b0)
    mmb[:, :, JC * N:] = chunkT(b1)

    in_maps = []
    for c in range(NCORES):
        s = slice(c * BPC, (c + 1) * BPC)
        in_maps.append({"bina": bina[s], "binb": binb[s], "mm": mmb[s]})
    return in_maps


def run(inputs, trace=False, **spmd_kwargs):
    """Full pipeline; returns (output, BassKernelResults)."""
    from concourse import bass_utils

    in_maps = _host_prep(**inputs)
    nc = _get_nc()
    res = bass_utils.run_bass_kernel_spmd(
        nc, in_maps, core_ids=list(range(NCORES)), trace=trace, **spmd_kwargs
    )
    outs = []
    for r in res.results:
        o = np.asarray(r["out"], np.float32)          # [BPC, 128, IC, DIM]
        outs.append(o.transpose(0, 2, 1, 3).reshape(BPC, N, DIM))
    full = np.concatenate(outs, axis=0)
    return full, res


def kernel(**inputs) -> np.ndarray:
    out, _ = run(inputs, trace=False)
    return out

